# revision 31
# baseline (speedup 1.0000x reference)
"""GCN2 layer (message passing + initial residual + 64x64 linear + relu)
on 8 Trainium2 NeuronCores via Bass/Tile.

v2 strategy (bf16, dest-sharded graph parallel):
  - Degree: host packs per-node incident weights into a dense [node, DW]
    table; deg = 1 + row-sum on DVE (no matmuls).
  - y = x*dinv computed in bf16, padded to 128 channels (256B rows) and
    AllGathered in 4 chunks (chunk == gather bank).
  - Messages: edges sharded by dest core, cells = (dest-window of 128,
    src-bank of 28672 rows); per-cell chunk counts = max over the 8 cores
    (SPMD shared). dma_gather fetches y rows bf16 directly; weight scale
    on DVE; one-hot (iota==rlm) bf16 per window; accumulate via bf16
    matmuls into PSUM.
  - Epilogue batched: h = agg*0.9*dinv + x01*(1+9/deg); PE transposes;
    W1 applied with 512-wide moving operands; relu; transpose back.
"""

import numpy as np

N, E, C, M = 100000, 1200000, 64, 8
NL = 12500                 # real nodes per core
WIN = 128
NWY = 98                   # windows covering real nodes (x/y/out)
NW = 112                   # message-phase windows (padded node space)
NLP = NW * WIN             # 14336 padded nodes per core
NBANK = 4
# real-row collective chunks (uneven; chunk == gather bank, int16-safe)
CH = [3584, 3584, 3584, 1748]          # rows per core per chunk
CST = [0, 3584, 7168, 10752]           # chunk starts within a core
BSZ = [8 * c for c in CH]              # bank sizes in yf
BST = [0, 28672, 57344, 86016]         # bank starts in yf
NPR = 8 * NL               # 100000 real rows in yf
G = 4                      # windows per gather group
NG = NW // G               # gather groups
ALPHA = 0.1

_CACHE = {}


def _bf16():
    import concourse.mybir as mybir
    return mybir.dt.np(mybir.dt.bfloat16)


def _host_prep(x, edge_index, edge_weight):
    npbf = _bf16()
    row = np.asarray(edge_index[0], dtype=np.int64)   # dest
    col = np.asarray(edge_index[1], dtype=np.int64)   # src
    w = np.asarray(edge_weight, dtype=np.float32)

    # global source row in yf (uneven chunk-major layout, real rows only)
    lc = col % NL
    ck = np.minimum(lc // 3584, 3)
    chv = np.asarray(CH)[ck]
    cstv = np.asarray(CST)[ck]
    bank = ck
    bidx = ((col // NL) * chv + (lc - cstv)).astype(np.int32)

    dcore = row // NL
    NCELL = NW * NBANK

    # ---- per-core edge streams, cell = (dest window, src bank) ----
    cores = []
    cnts = np.zeros((M, NCELL), np.int64)
    for m in range(M):
        sel = np.nonzero(dcore == m)[0]
        rl = row[sel] % NL
        wwin = rl // WIN
        rloc = (rl % WIN).astype(np.float32)
        cell = wwin * NBANK + bank[sel]
        order = np.argsort(cell, kind="stable")
        eidx = sel[order]
        cs = cell[order]
        cnts[m] = np.bincount(cs, minlength=NCELL)
        cores.append((eidx, cs, rloc[order]))

    maxcnt = cnts.max(axis=0)                      # per cell, max over cores
    S_cell = -(-maxcnt // WIN)                     # chunks per cell (0 ok)
    sizes = S_cell * WIN
    S_wb = S_cell.reshape(NW, NBANK)
    nch_w = S_wb.sum(axis=1)                       # chunks per window
    REP = int(max(1, nch_w.max()))
    TOTCH = int(nch_w.sum())
    TOT = TOTCH * WIN

    # window-major padded slot offsets (cell order: w-major, b within)
    cell_off = np.concatenate([[0], np.cumsum(sizes)[:-1]])
    cum_nch = np.concatenate([[0], np.cumsum(nch_w)[:-1]])  # oh col offset

    # call order: (group g, bank b) -> cells (w in g, b)
    # chunk offset of cell (w,b) inside its call tile:
    call_meta = []        # per (g,b): (idx_off_slots, n_chunks)
    tile_chunk_off = np.zeros((NW, NBANK), np.int64)
    callperm_cells = []   # cell ids in call order
    off_slots = 0
    MC = 0
    for g in range(NG):
        for b in range(NBANK):
            c0 = 0
            for wdw in range(g * G, (g + 1) * G):
                tile_chunk_off[wdw, b] = c0
                c0 += S_wb[wdw, b]
                callperm_cells.append(wdw * NBANK + b)
            call_meta.append((off_slots, int(c0)))
            off_slots += int(c0) * WIN
            MC = max(MC, int(c0))
    assert off_slots == TOT

    # per-window matmul chunk list: (bank, tile_chunk_index)
    win_chunks = []
    for wdw in range(NW):
        lst = []
        for b in range(NBANK):
            for s in range(int(S_wb[wdw, b])):
                lst.append((b, int(tile_chunk_off[wdw, b]) + s))
        win_chunks.append(lst)

    # scatter edges into window-major slot space, then permute to call order
    call_slot_base = np.zeros(NCELL, np.int64)
    pos = 0
    for cid in callperm_cells:
        call_slot_base[cid] = pos
        pos += sizes[cid]

    msg_in = []
    for m in range(M):
        eidx, cs, rloc_s = cores[m]
        starts = np.concatenate([[0], np.cumsum(cnts[m])[:-1]])
        p_in_cell = np.arange(len(cs)) - starts[cs]
        slot_w = cell_off[cs] + p_in_cell          # window-major slot
        slot_c = call_slot_base[cs] + p_in_cell    # call-order slot

        rlm_arr = np.full(TOT, -1.0, np.float32)
        rlm_arr[slot_w] = rloc_s
        wm_arr = np.zeros(TOT, np.float32)
        wm_arr[slot_c] = w[eidx]
        idx_arr = np.zeros(TOT, np.int32)
        idx_arr[slot_c] = bidx[eidx]

        rlm2 = rlm_arr.reshape(TOTCH, WIN).T.astype(npbf).copy()   # [128,TOTCH]
        wm2 = wm_arr.reshape(TOTCH, WIN).T.astype(npbf).copy()
        idx2 = np.tile(
            idx_arr.astype(np.int16).reshape(TOT // 16, 16).T, (8, 1)
        ).copy()                                                   # [128,TOT/16]
        msg_in.append((idx2, wm2, rlm2))

    # ---- dense degree table (keyed by src node = col) ----
    score = col // NL
    deg_cnt = np.zeros((M, NL), np.int64)
    for m in range(M):
        sel = np.nonzero(score == m)[0]
        deg_cnt[m] = np.bincount(lc[sel], minlength=NL)
    DW = int(deg_cnt.max())
    DW = (DW + 3) // 4 * 4

    deg_in = []
    for m in range(M):
        sel = np.nonzero(score == m)[0]
        node = lc[sel]
        order = np.argsort(node, kind="stable")
        node_s = node[order]
        w_s = w[sel][order]
        starts = np.concatenate([[0], np.cumsum(deg_cnt[m])[:-1]])
        k = np.arange(len(node_s)) - starts[node_s]
        tab = np.zeros((NWY * WIN, DW), np.float32)
        tab[node_s, k] = w_s
        # [node, k] -> [p, w*DW + k]
        dw_t = tab.reshape(NWY, WIN, DW).transpose(1, 0, 2).reshape(WIN, NWY * DW)
        deg_in.append(dw_t.astype(npbf).copy())

    # ---- x shards (pre-scaled by ALPHA, bf16) ----
    x = np.asarray(x, dtype=np.float32)
    x01 = []
    for m in range(M):
        xm = np.zeros((NWY * WIN, C), np.float32)
        xm[:NL] = ALPHA * x[m * NL:(m + 1) * NL]
        x01.append(xm.astype(npbf))

    struct = (REP, TOTCH, TOT, MC, DW, tuple(nch_w.tolist()),
              tuple(int(S_wb[w_, b_]) for w_ in range(NW) for b_ in range(NBANK)))
    return struct, call_meta, win_chunks, cum_nch, msg_in, deg_in, x01


def _build(struct, call_meta, win_chunks, cum_nch):
    from concourse import bacc, tile
    from concourse.bass import MemorySpace
    import concourse.mybir as mybir

    REP, TOTCH, TOT, MC, DW, nch_w, _ = struct
    f32 = mybir.dt.float32
    bf16 = mybir.dt.bfloat16
    i16 = mybir.dt.int16
    eq = mybir.AluOpType.is_equal
    mult = mybir.AluOpType.mult
    add = mybir.AluOpType.add

    nc = bacc.Bacc("TRN2", target_bir_lowering=False, debug=False,
                   num_devices=M, num_swdge_queues=4)

    x01_d = nc.dram_tensor("x01", [NWY * WIN, C], bf16, kind="ExternalInput")
    iota_d = nc.dram_tensor("iota", [WIN, REP * WIN], bf16, kind="ExternalInput")
    id_d = nc.dram_tensor("idm", [WIN, WIN], bf16, kind="ExternalInput")
    w1_d = nc.dram_tensor("w1", [C, C], bf16, kind="ExternalInput")
    dw_d = nc.dram_tensor("dw", [WIN, NWY * DW], bf16, kind="ExternalInput")
    wm_d = nc.dram_tensor("wm", [WIN, TOTCH], bf16, kind="ExternalInput")
    rlm_d = nc.dram_tensor("rlm", [WIN, TOTCH], bf16, kind="ExternalInput")
    idx_d = nc.dram_tensor("idxm", [128, TOT // 16], i16, kind="ExternalInput")
    out_d = nc.dram_tensor("out", [NWY * WIN, C], f32, kind="ExternalOutput")
    y_d = nc.dram_tensor("y_sh", [NL, 128], bf16)
    yf_d = nc.dram_tensor("y_full", [NPR, 128], bf16, addr_space="Shared")

    with tile.TileContext(nc) as tc:
        with (
            tc.tile_pool(name="res", bufs=1) as res,
            tc.tile_pool(name="work", bufs=4) as work,
            tc.tile_pool(name="msg", bufs=4) as msgp,
            tc.tile_pool(name="ps", bufs=2, space=MemorySpace.PSUM) as ps,
        ):
            # constants (host-provided; keeps Q7 free)
            id128 = res.tile([WIN, WIN], bf16)
            nc.sync.dma_start(out=id128[:], in_=id_d.ap())
            iota_t = res.tile([WIN, REP * WIN], bf16)
            nc.sync.dma_start(out=iota_t[:], in_=iota_d.ap())
            iota_c = iota_t[:].rearrange("p (r j) -> p r j", j=WIN)

            # degree table + x first: they gate y and thus the collectives
            dw_sb = res.tile([WIN, NWY * DW], bf16)
            nc.sync.dma_start(out=dw_sb[:], in_=dw_d.ap())
            x_sb = res.tile([WIN, NWY, C], bf16)
            nc.sync.dma_start(out=x_sb[:],
                              in_=x01_d.ap().rearrange("(w p) d -> p w d", p=WIN))
            w1_sb = res.tile([C, C], bf16)
            nc.sync.dma_start(out=w1_sb[:], in_=w1_d.ap())
            wm_sb = res.tile([WIN, TOTCH], bf16)
            nc.sync.dma_start(out=wm_sb[:], in_=wm_d.ap())
            rlm_sb = res.tile([WIN, TOTCH], bf16)
            nc.sync.dma_start(out=rlm_sb[:], in_=rlm_d.ap())
            idx_sb = res.tile([128, TOT // 16], i16)
            nc.sync.dma_start(out=idx_sb[:], in_=idx_d.ap())

            # ---- degree phase (dense table row-sum) ----
            deg = res.tile([WIN, NWY], f32)
            nc.vector.tensor_reduce(
                deg[:], dw_sb[:].rearrange("p (w k) -> p w k", k=DW),
                mybir.AxisListType.X, add)
            nc.vector.tensor_scalar_add(deg[:], deg[:], 1.0)
            rec = res.tile([WIN, NWY], f32)
            nc.vector.reciprocal(rec[:], deg[:])
            dinv = res.tile([WIN, NWY], f32)
            nc.scalar.sqrt(dinv[:], rec[:])
            dinv09 = res.tile([WIN, NWY], bf16)
            nc.vector.tensor_scalar_mul(dinv09[:], dinv[:], 0.9)
            dinv10 = res.tile([WIN, NWY], bf16)
            nc.vector.tensor_scalar_mul(dinv10[:], dinv[:], 10.0)
            c3 = res.tile([WIN, NWY], bf16)
            nc.vector.tensor_scalar(c3[:], rec[:], 9.0, 1.0, mult, add)

            # ---- y = x * dinv (bf16, zero-padded to 128 ch) ----
            ybuf = res.tile([WIN, NWY * 128], bf16)
            y_sb = ybuf[:].rearrange("p (w d) -> p w d", d=128)
            nc.vector.memset(ybuf[:], 0.0)
            nc.vector.tensor_tensor(
                y_sb[:, :NWY, 0:C], x_sb[:],
                dinv10[:].broadcast_to([WIN, NWY, C]), mult)
            # y_d holds only the 12500 real rows; window 97 is partial
            nc.sync.dma_start(
                out=y_d.ap()[0:97 * WIN, :].rearrange("(w p) d -> p w d", p=WIN),
                in_=y_sb[:, :97, :])
            nc.sync.dma_start(out=y_d.ap()[97 * WIN:NL, :],
                              in_=y_sb[0:NL - 97 * WIN, 97, :])
            for ckk in range(4):
                nc.gpsimd.collective_compute(
                    "AllGather", mybir.AluOpType.bypass,
                    replica_groups=[list(range(M))],
                    ins=[y_d.ap()[CST[ckk]:CST[ckk] + CH[ckk], :]],
                    outs=[yf_d.ap()[BST[ckk]:BST[ckk] + BSZ[ckk], :]])

            agg_all = res.tile([WIN, NWY, C], bf16)
            nc.vector.memset(agg_all[:], 0.0)
            # pre-scale x by (1 + 9/deg) now that y no longer needs raw x
            nc.vector.tensor_tensor(
                x_sb[:], x_sb[:], c3[:].broadcast_to([WIN, NWY, C]), mult)

            # ---- message phase ----
            yf_ap = yf_d.ap()
            for g in range(NG):
                mts = []
                for b in range(NBANK):
                    off_slots, nch = call_meta[g * NBANK + b]
                    if nch == 0:
                        mts.append(None)
                        continue
                    mt = msgp.tile([WIN, MC, 128], bf16, tag=f"m{b}")
                    nidx = nch * WIN
                    nc.gpsimd.dma_gather(
                        mt[:, :nch, :], yf_ap[BST[b]:BST[b] + BSZ[b], :],
                        idx_sb[:, off_slots // 16:(off_slots + nidx) // 16],
                        num_idxs=nidx, num_idxs_reg=nidx, elem_size=128,
                        single_packet=False, queue_num=(g + b) % 4)
                    cho = off_slots // WIN
                    nc.vector.tensor_tensor(
                        mt[:, :nch, 0:C], mt[:, :nch, 0:C],
                        wm_sb[:, cho:cho + nch].broadcast_to([WIN, nch, C]),
                        mult)
                    mts.append(mt)
                gwins = [w for w in range(g * G, (g + 1) * G)
                         if w < NWY and win_chunks[w]]
                for wdw in gwins:
                    chunks = win_chunks[wdw]
                    nch = len(chunks)
                    co = int(cum_nch[wdw])
                    oh = work.tile([WIN, REP, WIN], bf16, tag="oh")
                    nc.vector.tensor_tensor(
                        oh[:, :nch, :], iota_c[:, :nch, :],
                        rlm_sb[:, co:co + nch].broadcast_to([WIN, nch, WIN]),
                        eq)
                    aps = ps.tile([WIN, C], f32, tag="agg")
                    for t, (b, ci) in enumerate(chunks):
                        nc.tensor.matmul(aps[:], oh[:, t, :],
                                         mts[b][:, ci, 0:C],
                                         start=(t == 0), stop=(t == nch - 1))
                    nc.scalar.copy(agg_all[:, wdw, :], aps[:])

                # ---- inline epilogue for this group's windows ----
                if not gwins:
                    continue
                w0, w1e = gwins[0], gwins[-1] + 1
                nw = w1e - w0
                nc.vector.tensor_tensor(
                    agg_all[:, w0:w1e, :], agg_all[:, w0:w1e, :],
                    dinv09[:, w0:w1e].broadcast_to([WIN, nw, C]), mult)
                nc.vector.tensor_tensor(
                    agg_all[:, w0:w1e, :], agg_all[:, w0:w1e, :],
                    x_sb[:, w0:w1e, :], add)
                hT = work.tile([C, G * WIN], bf16, tag="hT")
                for k, wdw in enumerate(gwins):
                    hp = ps.tile([C, WIN], bf16, tag="hTp")
                    nc.tensor.transpose(hp[:], agg_all[:, wdw, :], id128[:])
                    nc.scalar.copy(hT[:, k * WIN:(k + 1) * WIN], hp[:])
                j = 0
                while j < nw * WIN:
                    je = min(j + 512, nw * WIN)
                    o2p = ps.tile([C, 512], f32, tag="o2")
                    nc.tensor.matmul(o2p[:, :je - j], w1_sb[:], hT[:, j:je],
                                     start=True, stop=True)
                    nc.scalar.activation(hT[:, j:je], o2p[:, :je - j],
                                         mybir.ActivationFunctionType.Relu)
                    j = je
                ot = work.tile([WIN, G, C], f32, tag="ot")
                for k, wdw in enumerate(gwins):
                    o3p = ps.tile([WIN, C], bf16, tag="o3")
                    nc.tensor.transpose(o3p[:], hT[:, k * WIN:(k + 1) * WIN],
                                        id128[:C, :C])
                    nc.scalar.copy(ot[:, k, :], o3p[:])
                nc.sync.dma_start(
                    out=out_d.ap()[w0 * WIN:w1e * WIN, :].rearrange(
                        "(w p) d -> p w d", p=WIN),
                    in_=ot[:, :nw, :])

    nc.compile()
    return nc


def kernel(x, edge_index, edge_weight, W1, _trace=False):
    from concourse.bass_utils import run_bass_kernel_spmd

    struct, call_meta, win_chunks, cum_nch, msg_in, deg_in, x01 = _host_prep(
        x, edge_index, edge_weight)
    key = struct
    if key not in _CACHE:
        _CACHE[key] = _build(struct, call_meta, win_chunks, cum_nch)
    nc = _CACHE[key]

    npbf = _bf16()
    w1 = np.asarray(W1, dtype=np.float32).astype(npbf)
    REP = struct[0]
    iota = np.tile(np.arange(WIN, dtype=np.float32).astype(npbf),
                   (WIN, REP)).copy()
    idm = np.eye(WIN, dtype=np.float32).astype(npbf)
    in_maps = []
    for m in range(M):
        idxm, wm, rlm = msg_in[m]
        in_maps.append({
            "x01": x01[m], "w1": w1, "dw": deg_in[m],
            "wm": wm, "rlm": rlm, "idxm": idxm,
            "iota": iota, "idm": idm,
        })
    res = run_bass_kernel_spmd(nc, in_maps, list(range(M)), trace=_trace)

    full = np.empty((N, C), np.float32)
    for m in range(M):
        full[m * NL:(m + 1) * NL] = res.results[m]["out"][:NL]
    if _trace:
        return full, res
    return full


# revision 33
# speedup vs baseline: 1.0104x; 1.0104x over previous
"""GCN2 layer (message passing + initial residual + 64x64 linear + relu)
on 8 Trainium2 NeuronCores via Bass/Tile.

v2 strategy (bf16, dest-sharded graph parallel):
  - Degree: host packs per-node incident weights into a dense [node, DW]
    table; deg = 1 + row-sum on DVE (no matmuls).
  - y = x*dinv computed in bf16, padded to 128 channels (256B rows) and
    AllGathered in 4 chunks (chunk == gather bank).
  - Messages: edges sharded by dest core, cells = (dest-window of 128,
    src-bank of 28672 rows); per-cell chunk counts = max over the 8 cores
    (SPMD shared). dma_gather fetches y rows bf16 directly; weight scale
    on DVE; one-hot (iota==rlm) bf16 per window; accumulate via bf16
    matmuls into PSUM.
  - Epilogue batched: h = agg*0.9*dinv + x01*(1+9/deg); PE transposes;
    W1 applied with 512-wide moving operands; relu; transpose back.
"""

import numpy as np

N, E, C, M = 100000, 1200000, 64, 8
NL = 12500                 # real nodes per core
WIN = 128
NWY = 98                   # windows covering real nodes (x/y/out)
NW = 112                   # message-phase windows (padded node space)
NLP = NW * WIN             # 14336 padded nodes per core
NBANK = 4
# real-row collective chunks (uneven; chunk == gather bank, int16-safe).
# The small chunk goes FIRST so the first AllGather lands quickly and
# bank-0 gathers can start early.
CH = [1748, 3584, 3584, 3584]          # rows per core per chunk
CST = [10752, 0, 3584, 7168]           # chunk starts (in local node id)
BSZ = [8 * c for c in CH]              # bank sizes in yf
BST = [0, 13984, 42656, 71328]         # bank starts in yf
NPR = 8 * NL               # 100000 real rows in yf
G = 4                      # windows per gather group
NG = NW // G               # gather groups
ALPHA = 0.1

_CACHE = {}


def _bf16():
    import concourse.mybir as mybir
    return mybir.dt.np(mybir.dt.bfloat16)


def _host_prep(x, edge_index, edge_weight):
    npbf = _bf16()
    row = np.asarray(edge_index[0], dtype=np.int64)   # dest
    col = np.asarray(edge_index[1], dtype=np.int64)   # src
    w = np.asarray(edge_weight, dtype=np.float32)

    # global source row in yf (uneven chunk-major layout, real rows only)
    lc = col % NL
    ck = np.where(lc >= 10752, 0, lc // 3584 + 1)
    chv = np.asarray(CH)[ck]
    cstv = np.asarray(CST)[ck]
    bank = ck
    bidx = ((col // NL) * chv + (lc - cstv)).astype(np.int32)

    dcore = row // NL
    NCELL = NW * NBANK

    # ---- per-core edge streams, cell = (dest window, src bank) ----
    cores = []
    cnts = np.zeros((M, NCELL), np.int64)
    for m in range(M):
        sel = np.nonzero(dcore == m)[0]
        rl = row[sel] % NL
        wwin = rl // WIN
        rloc = (rl % WIN).astype(np.float32)
        cell = wwin * NBANK + bank[sel]
        order = np.argsort(cell, kind="stable")
        eidx = sel[order]
        cs = cell[order]
        cnts[m] = np.bincount(cs, minlength=NCELL)
        cores.append((eidx, cs, rloc[order]))

    maxcnt = cnts.max(axis=0)                      # per cell, max over cores
    S_cell = -(-maxcnt // WIN)                     # chunks per cell (0 ok)
    sizes = S_cell * WIN
    S_wb = S_cell.reshape(NW, NBANK)
    nch_w = S_wb.sum(axis=1)                       # chunks per window
    REP = int(max(1, nch_w.max()))
    TOTCH = int(nch_w.sum())
    TOT = TOTCH * WIN

    # window-major padded slot offsets (cell order: w-major, b within)
    cell_off = np.concatenate([[0], np.cumsum(sizes)[:-1]])
    cum_nch = np.concatenate([[0], np.cumsum(nch_w)[:-1]])  # oh col offset

    # call order: (group g, bank b) -> cells (w in g, b)
    # chunk offset of cell (w,b) inside its call tile:
    call_meta = []        # per (g,b): (idx_off_slots, n_chunks)
    tile_chunk_off = np.zeros((NW, NBANK), np.int64)
    callperm_cells = []   # cell ids in call order
    off_slots = 0
    MC = 0
    for g in range(NG):
        for b in range(NBANK):
            c0 = 0
            for wdw in range(g * G, (g + 1) * G):
                tile_chunk_off[wdw, b] = c0
                c0 += S_wb[wdw, b]
                callperm_cells.append(wdw * NBANK + b)
            call_meta.append((off_slots, int(c0)))
            off_slots += int(c0) * WIN
            MC = max(MC, int(c0))
    assert off_slots == TOT

    # per-window matmul chunk list: (bank, tile_chunk_index)
    win_chunks = []
    for wdw in range(NW):
        lst = []
        for b in range(NBANK):
            for s in range(int(S_wb[wdw, b])):
                lst.append((b, int(tile_chunk_off[wdw, b]) + s))
        win_chunks.append(lst)

    # scatter edges into window-major slot space, then permute to call order
    call_slot_base = np.zeros(NCELL, np.int64)
    pos = 0
    for cid in callperm_cells:
        call_slot_base[cid] = pos
        pos += sizes[cid]

    msg_in = []
    for m in range(M):
        eidx, cs, rloc_s = cores[m]
        starts = np.concatenate([[0], np.cumsum(cnts[m])[:-1]])
        p_in_cell = np.arange(len(cs)) - starts[cs]
        slot_w = cell_off[cs] + p_in_cell          # window-major slot
        slot_c = call_slot_base[cs] + p_in_cell    # call-order slot

        rlm_arr = np.full(TOT, -1.0, np.float32)
        rlm_arr[slot_w] = rloc_s
        wm_arr = np.zeros(TOT, np.float32)
        wm_arr[slot_c] = w[eidx]
        idx_arr = np.zeros(TOT, np.int32)
        idx_arr[slot_c] = bidx[eidx]

        rlm2 = rlm_arr.reshape(TOTCH, WIN).T.astype(npbf).copy()   # [128,TOTCH]
        wm2 = wm_arr.reshape(TOTCH, WIN).T.astype(npbf).copy()
        idx2 = np.tile(
            idx_arr.astype(np.int16).reshape(TOT // 16, 16).T, (8, 1)
        ).copy()                                                   # [128,TOT/16]
        msg_in.append((idx2, wm2, rlm2))

    # ---- dense degree table (keyed by src node = col) ----
    score = col // NL
    deg_cnt = np.zeros((M, NL), np.int64)
    for m in range(M):
        sel = np.nonzero(score == m)[0]
        deg_cnt[m] = np.bincount(lc[sel], minlength=NL)
    DW = int(deg_cnt.max())
    DW = (DW + 3) // 4 * 4

    deg_in = []
    for m in range(M):
        sel = np.nonzero(score == m)[0]
        node = lc[sel]
        order = np.argsort(node, kind="stable")
        node_s = node[order]
        w_s = w[sel][order]
        starts = np.concatenate([[0], np.cumsum(deg_cnt[m])[:-1]])
        k = np.arange(len(node_s)) - starts[node_s]
        tab = np.zeros((NWY * WIN, DW), np.float32)
        tab[node_s, k] = w_s
        # [node, k] -> [p, w*DW + k]
        dw_t = tab.reshape(NWY, WIN, DW).transpose(1, 0, 2).reshape(WIN, NWY * DW)
        deg_in.append(dw_t.astype(npbf).copy())

    # ---- x shards (pre-scaled by ALPHA, bf16) ----
    x = np.asarray(x, dtype=np.float32)
    x01 = []
    for m in range(M):
        xm = np.zeros((NWY * WIN, C), np.float32)
        xm[:NL] = ALPHA * x[m * NL:(m + 1) * NL]
        x01.append(xm.astype(npbf))

    struct = (REP, TOTCH, TOT, MC, DW, tuple(nch_w.tolist()),
              tuple(int(S_wb[w_, b_]) for w_ in range(NW) for b_ in range(NBANK)))
    return struct, call_meta, win_chunks, cum_nch, msg_in, deg_in, x01


def _build(struct, call_meta, win_chunks, cum_nch):
    from concourse import bacc, tile
    from concourse.bass import MemorySpace
    import concourse.mybir as mybir

    REP, TOTCH, TOT, MC, DW, nch_w, _ = struct
    f32 = mybir.dt.float32
    bf16 = mybir.dt.bfloat16
    i16 = mybir.dt.int16
    eq = mybir.AluOpType.is_equal
    mult = mybir.AluOpType.mult
    add = mybir.AluOpType.add

    nc = bacc.Bacc("TRN2", target_bir_lowering=False, debug=False,
                   num_devices=M, num_swdge_queues=4)

    x01_d = nc.dram_tensor("x01", [NWY * WIN, C], bf16, kind="ExternalInput")
    iota_d = nc.dram_tensor("iota", [WIN, REP * WIN], bf16, kind="ExternalInput")
    id_d = nc.dram_tensor("idm", [WIN, WIN], bf16, kind="ExternalInput")
    w1_d = nc.dram_tensor("w1", [C, C], bf16, kind="ExternalInput")
    dw_d = nc.dram_tensor("dw", [WIN, NWY * DW], bf16, kind="ExternalInput")
    wm_d = nc.dram_tensor("wm", [WIN, TOTCH], bf16, kind="ExternalInput")
    rlm_d = nc.dram_tensor("rlm", [WIN, TOTCH], bf16, kind="ExternalInput")
    idx_d = nc.dram_tensor("idxm", [128, TOT // 16], i16, kind="ExternalInput")
    out_d = nc.dram_tensor("out", [NWY * WIN, C], f32, kind="ExternalOutput")
    y_d = nc.dram_tensor("y_sh", [NL, 128], bf16)
    yf_d = nc.dram_tensor("y_full", [NPR, 128], bf16, addr_space="Shared")

    with tile.TileContext(nc) as tc:
        with (
            tc.tile_pool(name="res", bufs=1) as res,
            tc.tile_pool(name="work", bufs=4) as work,
            tc.tile_pool(name="msg", bufs=4) as msgp,
            tc.tile_pool(name="ps", bufs=2, space=MemorySpace.PSUM) as ps,
        ):
            # constants (host-provided; keeps Q7 free)
            id128 = res.tile([WIN, WIN], bf16)
            nc.sync.dma_start(out=id128[:], in_=id_d.ap())
            iota_t = res.tile([WIN, REP * WIN], bf16)
            nc.sync.dma_start(out=iota_t[:], in_=iota_d.ap())
            iota_c = iota_t[:].rearrange("p (r j) -> p r j", j=WIN)

            # degree table + x first: they gate y and thus the collectives
            dw_sb = res.tile([WIN, NWY * DW], bf16)
            nc.sync.dma_start(out=dw_sb[:], in_=dw_d.ap())
            x_sb = res.tile([WIN, NWY, C], bf16)
            nc.sync.dma_start(out=x_sb[:],
                              in_=x01_d.ap().rearrange("(w p) d -> p w d", p=WIN))
            w1_sb = res.tile([C, C], bf16)
            nc.sync.dma_start(out=w1_sb[:], in_=w1_d.ap())
            wm_sb = res.tile([WIN, TOTCH], bf16)
            nc.sync.dma_start(out=wm_sb[:], in_=wm_d.ap())
            rlm_sb = res.tile([WIN, TOTCH], bf16)
            nc.sync.dma_start(out=rlm_sb[:], in_=rlm_d.ap())
            idx_sb = res.tile([128, TOT // 16], i16)
            nc.sync.dma_start(out=idx_sb[:], in_=idx_d.ap())

            # ---- degree phase (dense table row-sum) ----
            deg = res.tile([WIN, NWY], f32)
            nc.vector.tensor_reduce(
                deg[:], dw_sb[:].rearrange("p (w k) -> p w k", k=DW),
                mybir.AxisListType.X, add)
            nc.vector.tensor_scalar_add(deg[:], deg[:], 1.0)
            rec = res.tile([WIN, NWY], f32)
            nc.vector.reciprocal(rec[:], deg[:])
            dinv = res.tile([WIN, NWY], f32)
            nc.scalar.sqrt(dinv[:], rec[:])
            dinv09 = res.tile([WIN, NWY], bf16)
            nc.vector.tensor_scalar_mul(dinv09[:], dinv[:], 0.9)
            dinv10 = res.tile([WIN, NWY], bf16)
            nc.vector.tensor_scalar_mul(dinv10[:], dinv[:], 10.0)
            c3 = res.tile([WIN, NWY], bf16)
            nc.vector.tensor_scalar(c3[:], rec[:], 9.0, 1.0, mult, add)

            # ---- y = x * dinv (bf16, zero-padded to 128 ch) ----
            ybuf = res.tile([WIN, NWY * 128], bf16)
            y_sb = ybuf[:].rearrange("p (w d) -> p w d", d=128)
            nc.vector.memset(ybuf[:], 0.0)
            nc.vector.tensor_tensor(
                y_sb[:, :NWY, 0:C], x_sb[:],
                dinv10[:].broadcast_to([WIN, NWY, C]), mult)
            # y_d holds only the 12500 real rows; window 97 is partial
            nc.sync.dma_start(
                out=y_d.ap()[0:97 * WIN, :].rearrange("(w p) d -> p w d", p=WIN),
                in_=y_sb[:, :97, :])
            nc.sync.dma_start(out=y_d.ap()[97 * WIN:NL, :],
                              in_=y_sb[0:NL - 97 * WIN, 97, :])
            for ckk in range(4):
                nc.gpsimd.collective_compute(
                    "AllGather", mybir.AluOpType.bypass,
                    replica_groups=[list(range(M))],
                    ins=[y_d.ap()[CST[ckk]:CST[ckk] + CH[ckk], :]],
                    outs=[yf_d.ap()[BST[ckk]:BST[ckk] + BSZ[ckk], :]])

            agg_all = res.tile([WIN, NWY, C], bf16)
            nc.vector.memset(agg_all[:], 0.0)
            # pre-scale x by (1 + 9/deg) now that y no longer needs raw x
            nc.vector.tensor_tensor(
                x_sb[:], x_sb[:], c3[:].broadcast_to([WIN, NWY, C]), mult)

            # ---- message phase ----
            yf_ap = yf_d.ap()
            for g in range(NG):
                mts = []
                for b in range(NBANK):
                    off_slots, nch = call_meta[g * NBANK + b]
                    if nch == 0:
                        mts.append(None)
                        continue
                    mt = msgp.tile([WIN, MC, 128], bf16, tag=f"m{b}")
                    nidx = nch * WIN
                    nc.gpsimd.dma_gather(
                        mt[:, :nch, :], yf_ap[BST[b]:BST[b] + BSZ[b], :],
                        idx_sb[:, off_slots // 16:(off_slots + nidx) // 16],
                        num_idxs=nidx, num_idxs_reg=nidx, elem_size=128,
                        single_packet=False, queue_num=(g + b) % 4)
                    cho = off_slots // WIN
                    nc.vector.tensor_tensor(
                        mt[:, :nch, 0:C], mt[:, :nch, 0:C],
                        wm_sb[:, cho:cho + nch].broadcast_to([WIN, nch, C]),
                        mult)
                    mts.append(mt)
                gwins = [w for w in range(g * G, (g + 1) * G)
                         if w < NWY and win_chunks[w]]
                for wdw in gwins:
                    chunks = win_chunks[wdw]
                    nch = len(chunks)
                    co = int(cum_nch[wdw])
                    oh = work.tile([WIN, REP, WIN], bf16, tag="oh")
                    nc.vector.tensor_tensor(
                        oh[:, :nch, :], iota_c[:, :nch, :],
                        rlm_sb[:, co:co + nch].broadcast_to([WIN, nch, WIN]),
                        eq)
                    aps = ps.tile([WIN, C], f32, tag="agg")
                    for t, (b, ci) in enumerate(chunks):
                        nc.tensor.matmul(aps[:], oh[:, t, :],
                                         mts[b][:, ci, 0:C],
                                         start=(t == 0), stop=(t == nch - 1))
                    nc.scalar.copy(agg_all[:, wdw, :], aps[:])

                # ---- inline epilogue for this group's windows ----
                if not gwins:
                    continue
                w0, w1e = gwins[0], gwins[-1] + 1
                nw = w1e - w0
                nc.vector.tensor_tensor(
                    agg_all[:, w0:w1e, :], agg_all[:, w0:w1e, :],
                    dinv09[:, w0:w1e].broadcast_to([WIN, nw, C]), mult)
                nc.vector.tensor_tensor(
                    agg_all[:, w0:w1e, :], agg_all[:, w0:w1e, :],
                    x_sb[:, w0:w1e, :], add)
                hT = work.tile([C, G * WIN], bf16, tag="hT")
                for k, wdw in enumerate(gwins):
                    hp = ps.tile([C, WIN], bf16, tag="hTp")
                    nc.tensor.transpose(hp[:], agg_all[:, wdw, :], id128[:])
                    nc.scalar.copy(hT[:, k * WIN:(k + 1) * WIN], hp[:])
                j = 0
                while j < nw * WIN:
                    je = min(j + 512, nw * WIN)
                    o2p = ps.tile([C, 512], f32, tag="o2")
                    nc.tensor.matmul(o2p[:, :je - j], w1_sb[:], hT[:, j:je],
                                     start=True, stop=True)
                    nc.scalar.activation(hT[:, j:je], o2p[:, :je - j],
                                         mybir.ActivationFunctionType.Relu)
                    j = je
                ot = work.tile([WIN, G, C], f32, tag="ot")
                for k, wdw in enumerate(gwins):
                    o3p = ps.tile([WIN, C], bf16, tag="o3")
                    nc.tensor.transpose(o3p[:], hT[:, k * WIN:(k + 1) * WIN],
                                        id128[:C, :C])
                    nc.scalar.copy(ot[:, k, :], o3p[:])
                nc.sync.dma_start(
                    out=out_d.ap()[w0 * WIN:w1e * WIN, :].rearrange(
                        "(w p) d -> p w d", p=WIN),
                    in_=ot[:, :nw, :])

    nc.compile()
    return nc


def kernel(x, edge_index, edge_weight, W1, _trace=False):
    from concourse.bass_utils import run_bass_kernel_spmd

    struct, call_meta, win_chunks, cum_nch, msg_in, deg_in, x01 = _host_prep(
        x, edge_index, edge_weight)
    key = struct
    if key not in _CACHE:
        _CACHE[key] = _build(struct, call_meta, win_chunks, cum_nch)
    nc = _CACHE[key]

    npbf = _bf16()
    w1 = np.asarray(W1, dtype=np.float32).astype(npbf)
    REP = struct[0]
    iota = np.tile(np.arange(WIN, dtype=np.float32).astype(npbf),
                   (WIN, REP)).copy()
    idm = np.eye(WIN, dtype=np.float32).astype(npbf)
    in_maps = []
    for m in range(M):
        idxm, wm, rlm = msg_in[m]
        in_maps.append({
            "x01": x01[m], "w1": w1, "dw": deg_in[m],
            "wm": wm, "rlm": rlm, "idxm": idxm,
            "iota": iota, "idm": idm,
        })
    res = run_bass_kernel_spmd(nc, in_maps, list(range(M)), trace=_trace)

    full = np.empty((N, C), np.float32)
    for m in range(M):
        full[m * NL:(m + 1) * NL] = res.results[m]["out"][:NL]
    if _trace:
        return full, res
    return full
